# revision 12
# baseline (speedup 1.0000x reference)
"""Trainium2 Bass kernel for nn_ANModel (gnn_message_passing), 8-core SPMD.

kernel(**inputs) takes the FULL unsharded inputs (as produced by the
reference setup_inputs()) and returns the full (B=8, M=6) float32 output.

Strategy (one batch element per NeuronCore, no collectives):
  The reference's graph-conv loop computes res/agg from x (not x_adapt) in
  both iterations, so only the second iteration's weights reach the output;
  and the output reads x_adapt only at the M=6 rows selected by
  road_neighbor_idxs[ridxs[b]], so only those rows are computed.

  Host-side prep (sharding/gather/layout packing + parameter-only folds):
  the 6 needed edge/adjacency/obs_all rows are gathered on the host into
  two contiguous per-core tensors (bulk data + mask-folded gated/MLP
  weights in bf16, the small fp32 weight folds separately), and every
  weight-only product is precomputed in numpy.  The device runs two direct
  DMAs in (issued from the ACT engine's HW-DGE queue, which reaches its
  program earlier than the sync engine), the batch-data compute (edge
  reduce on Vector, neighborhood/fusion/MLP matmuls on PE, sigmoid/relu),
  and one DMA out — no indirect DMA, no transposes, no weight-fold matmuls
  on the critical path.
"""
import os
import sys

import numpy as np

try:
    import concourse.bass as bass
except ImportError:
    sys.path.insert(0, "/opt/trn_rl_repo")
    import concourse.bass as bass

import concourse.tile as tile
from concourse import mybir

# ---------------------------------------------------------------------------
# Workarounds for walrus builds that support only ONE sync-wait/instruction.

import concourse.tile as tile
import concourse.bass_interp as bass_interp
from concourse import mybir
from concourse.vector_clock import ScopedClock
from concourse.tile_sem_assignment import TileClockWait as _RealTCW


def _patched_drain_and_barrier(self, tick_clock, wait_clock):
    probe = self.nc.sync.drain()
    wait_clock.add_sem_waits(probe.ins, ScopedClock({None: tick_clock.global_clock}))
    si = probe.ins.sync_info
    waits = list(si.on_wait) if si and si.on_wait else []
    if len(waits) > 1:
        si.on_wait = [waits[0]]
        for w in waits[1:]:
            d = self.nc.sync.drain()
            dsi = d.ins.sync_info
            if dsi is None:
                d.ins.sync_info = mybir.SyncInfo(on_wait=[w], on_update=[])
            else:
                dsi.on_wait = [w]
    slim = os.environ.get("BASSFIX_SLIM_TAIL", "0") == "1"
    self.nc.all_engine_barrier()
    popped = self.nc._tile_sem_poison_stack.pop()
    assert popped is self._sem_poison
    self.nc.clear_and_free_semaphores(list(self.sems.allocated().values()))
    if not slim:
        self.nc.all_engine_barrier()


class _SplitWaitTCW:
    def __init__(self, tc, blocks):
        self._tc = tc
        self._blocks = blocks
        self._inner = _RealTCW(tc, blocks)

    def assign_waits(self, bb_name):
        r = self._inner.assign_waits(bb_name)
        nc = self._tc.nc
        Op = nc.isa.Opcode
        for _name, insts in self._blocks.items():
            out = []
            changed = False
            for inst in insts:
                si = getattr(inst, "sync_info", None)
                if si is not None and si.on_wait and len(si.on_wait) > 1:
                    waits = list(si.on_wait)
                    si.on_wait = [waits[-1]]
                    for w in waits[:-1]:
                        eng = nc.engines[inst.engine]
                        nop = eng._isa(Op.NEURON_ISA_TPB_OPCODE_NOP, {})
                        nop.sync_info = mybir.SyncInfo(on_wait=[w], on_update=[])
                        out.append(nop)
                    changed = True
                out.append(inst)
            if changed:
                insts[:] = out
        return r

    def add_sem_waits(self, *a, **k):
        return self._inner.add_sem_waits(*a, **k)

    def __getattr__(self, k):
        return getattr(self._inner, k)


_real_visit_isa = bass_interp._visit_InstISA


def _patched_visit_isa(isa, instruction, core_sim):
    # Treat the sequencer NOP (used by _SplitWaitTCW as a wait carrier) as a no-op.
    if instruction.isa_opcode == isa.Opcode.NEURON_ISA_TPB_OPCODE_NOP.value:
        return None
    return _real_visit_isa(isa, instruction, core_sim)


def apply():
    tile.TileContext._drain_and_barrier = _patched_drain_and_barrier
    tile.TileClockWait = _SplitWaitTCW
    bass_interp._visit_InstISA = _patched_visit_isa


apply()

# ---------------------------------------------------------------------------
# Kernel builder

F32 = mybir.dt.float32
BF16 = mybir.dt.bfloat16
AX = mybir.AxisListType
OP = mybir.AluOpType
ACT = mybir.ActivationFunctionType

N, M, ED, SRC, OBS, D, H = 512, 6, 8, 64, 32, 16, 64
NQ = 16                  # j-chunks for the edge reduce: 6*16 = 96 partitions
P96 = M * NQ             # 96
JW = N // NQ             # 32 j's per chunk
EQW = ED * JW            # 256 cols of es96 per partition row


def _mk_cols(blocks):
    cols = {}
    c = 0
    for nm, r, cw in blocks:
        cols[nm] = (c, r, cw)
        c += cw
    return cols, c


# bf16 tensor 1: the edge block (gates the Vector reduce -> lands first)
_ECOLS, EKC = _mk_cols([
    ("es96", P96, EQW),        # edge rows, (m,q) x (e,j2) layout
    ("adj96", P96, JW),        # adj rows, (m,q) x j2
])
# bf16 tensor 2: remaining bulk data + mask-folded gated weights + MLP weights
_HCOLS, HKC = _mk_cols([
    ("oa4", 128, 4 * OBS),     # obs_all[b] as 4 chunks of (128, 32)
    ("adjT", 128, 4 * M),      # adj rows transposed, j2 x (q,m)
    ("gwm1", D, M * D),        # mask-scaled gated weights, "gated" half
    ("gwm2", D, M * D),        # mask-scaled gated weights, "gate" half
    ("bw1", D, H),
    ("bw2", H, H),
    ("awb", H + 1, M),         # [act_w; act_b]
])
# f32 tensor: small weight folds + batch scalars
_FCOLS, FKC = _mk_cols([
    ("s96", P96, M),           # (m,q) -> m selector
    ("hp66", 66, M),           # [oarT; oarT*deg; ones; deg]
    ("Wst", 66, D),            # [Wr; Wf1; cres; bias_deg]
    ("Wf2", OBS, D),
    ("Cw", ED, D),
    ("Wfold1", SRC + 1, D),    # [w_src @ gw_a | cg]  "gated" half
    ("Wfold2", SRC + 1, D),    # "gate" half
    ("obs1", SRC + 1, 1),      # [obs; 1]
    ("bb1", H, 1),
    ("bb2", H, 1),
])


def build_nc():
    nc = bass.Bass("TRN2", target_bir_lowering=False, debug=False)

    pe_d = nc.declare_dram_parameter("pe", [P96, EKC], BF16, isOutput=False)
    ph_d = nc.declare_dram_parameter("ph", [128, HKC], BF16, isOutput=False)
    pf_d = nc.declare_dram_parameter("pf", [128, FKC], F32, isOutput=False)
    out_d = nc.declare_dram_parameter("out", [M, 1], F32, isOutput=True)

    with tile.TileContext(nc) as tc:
        with (
            tc.tile_pool(name="sb", bufs=1) as sb,
            tc.tile_pool(name="acc", bufs=1, space="PSUM") as acc,
        ):
            pe = sb.tile([P96, EKC], BF16, tag="pe")
            ph = sb.tile([128, HKC], BF16, tag="ph")
            pf = sb.tile([128, FKC], F32, tag="pf")
            # one input DMA per engine queue so the transfers overlap:
            # edge block via sync, bulk bf16 via ACT HW-DGE, small f32 via
            # the (otherwise idle) GpSimd software DGE.
            nc.sync.dma_start(out=pe[:], in_=pe_d[:])
            nc.scalar.dma_start(out=ph[:], in_=ph_d[:])
            nc.gpsimd.dma_start(out=pf[:], in_=pf_d[:])

            def E_(name):
                c0, r, cw = _ECOLS[name]
                return pe[0:r, c0:c0 + cw]

            def H_(name):
                c0, r, cw = _HCOLS[name]
                return ph[0:r, c0:c0 + cw]

            def F_(name):
                c0, r, cw = _FCOLS[name]
                return pf[0:r, c0:c0 + cw]

            # ACT warm-up: trigger the activation table load while the input
            # DMAs are in flight (else the first ACT op pays ~1.3us on the
            # critical path).
            warm = sb.tile([1, 1], F32, tag="warm")
            nc.gpsimd.memset(warm[:], 0.0)
            warm2 = sb.tile([1, 1], F32, tag="warm2")
            nc.scalar.activation(out=warm2[:], in_=warm[:], func=ACT.Sigmoid)

            # h3 gets a constant-1 row appended so the action head is a
            # single fused [act_w; act_b] matmul.
            h3 = sb.tile([H + 1, 1], BF16, tag="h3")
            nc.gpsimd.memset(h3[H:H + 1, 0:1], 1.0)

            es96 = E_("es96")
            adj96 = E_("adj96")

            # ---- edge reduce: er[(m,q), e] = sum_j2 es*adj ----
            prod = sb.tile([P96, EQW], F32, tag="prod")
            in0 = bass.AP(tensor=es96.tensor, offset=es96.offset,
                          ap=[es96.ap[0], [JW, ED], [1, JW]])
            in1 = bass.AP(tensor=adj96.tensor, offset=adj96.offset,
                          ap=[adj96.ap[0], [0, ED], [1, JW]])
            out0 = bass.AP(tensor=prod[:].tensor, offset=prod[:].offset,
                           ap=[prod[:].ap[0], [JW, ED], [1, JW]])
            nc.vector.tensor_tensor(out=out0, in0=in0, in1=in1, op=OP.mult)
            erq = sb.tile([P96, ED], F32, tag="erq")
            pr = prod[:]
            red_in = bass.AP(tensor=pr.tensor, offset=pr.offset,
                             ap=[pr.ap[0], [JW, ED], [1, JW]])
            nc.vector.tensor_reduce(out=erq[:], in_=red_in, axis=AX.X, op=OP.add)

            # ---- PE: gated-fusion head start, S^T, er^T, zf^T, MLP ----
            g1_p = acc.tile([D, 1], F32, tag="g1_p")
            g2_p = acc.tile([D, 1], F32, tag="g2_p")
            nc.tensor.matmul(out=g1_p[:], lhsT=F_("Wfold1"), rhs=F_("obs1"),
                             start=True, stop=False)
            nc.tensor.matmul(out=g2_p[:], lhsT=F_("Wfold2"), rhs=F_("obs1"),
                             start=True, stop=False)
            zfT_p = acc.tile([D, M], F32, tag="zfT_p")
            nc.tensor.matmul(out=zfT_p[:], lhsT=F_("Wst"), rhs=F_("hp66"),
                             start=True, stop=False)
            sT_p = acc.tile([OBS, M], F32, tag="sT_p")
            oa4 = H_("oa4")
            adjT = H_("adjT")
            for c in range(4):
                nc.tensor.matmul(out=sT_p[:], lhsT=oa4[:, c * OBS:(c + 1) * OBS],
                                 rhs=adjT[:, c * M:(c + 1) * M],
                                 start=(c == 0), stop=(c == 3))
            erT_p = acc.tile([ED, M], F32, tag="erT_p")
            nc.tensor.matmul(out=erT_p[:], lhsT=erq[:], rhs=F_("s96"),
                             start=True, stop=True)
            sT = sb.tile([OBS, M], F32, tag="sT")
            nc.vector.tensor_copy(out=sT[:], in_=sT_p[:])
            erT = sb.tile([ED, M], F32, tag="erT")
            nc.scalar.copy(out=erT[:], in_=erT_p[:])
            nc.tensor.matmul(out=zfT_p[:], lhsT=F_("Wf2"), rhs=sT[:],
                             start=False, stop=False)
            nc.tensor.matmul(out=zfT_p[:], lhsT=F_("Cw"), rhs=erT[:],
                             start=False, stop=True)
            rT = sb.tile([D, M], BF16, tag="rT")
            nc.vector.tensor_scalar(out=rT[:], in0=zfT_p[:], scalar1=0.0,
                                    scalar2=None, op0=OP.max)
            gwm1 = H_("gwm1")
            gwm2 = H_("gwm2")
            for m in range(M):
                nc.tensor.matmul(out=g2_p[:], lhsT=gwm2[:, m * D:(m + 1) * D],
                                 rhs=rT[:, m:m + 1],
                                 start=False, stop=(m == M - 1))
            for m in range(M):
                nc.tensor.matmul(out=g1_p[:], lhsT=gwm1[:, m * D:(m + 1) * D],
                                 rhs=rT[:, m:m + 1],
                                 start=False, stop=(m == M - 1))
            sig = sb.tile([D, 1], F32, tag="sig")
            nc.scalar.activation(out=sig[:], in_=g2_p[:], func=ACT.Sigmoid)
            h = sb.tile([D, 1], BF16, tag="h")
            nc.vector.tensor_mul(out=h[:], in0=g1_p[:], in1=sig[:])

            h2_p = acc.tile([H, 1], F32, tag="h2_p")
            nc.tensor.matmul(out=h2_p[:], lhsT=H_("bw1"), rhs=h[:], start=True, stop=True)
            h2 = sb.tile([H, 1], BF16, tag="h2")
            nc.vector.tensor_scalar(out=h2[:], in0=h2_p[:], scalar1=F_("bb1")[:, 0:1],
                                    scalar2=0.0, op0=OP.add, op1=OP.max)
            h3_p = acc.tile([H, 1], F32, tag="h3_p")
            nc.tensor.matmul(out=h3_p[:], lhsT=H_("bw2"), rhs=h2[:], start=True, stop=True)
            nc.vector.tensor_scalar(out=h3[0:H, 0:1], in0=h3_p[:],
                                    scalar1=F_("bb2")[:, 0:1],
                                    scalar2=0.0, op0=OP.add, op1=OP.max)
            o_p = acc.tile([M, 1], F32, tag="o_p")
            nc.tensor.matmul(out=o_p[:], lhsT=H_("awb"), rhs=h3[:], start=True, stop=True)
            o_sb = sb.tile([M, 1], F32, tag="o_sb")
            nc.scalar.copy(out=o_sb[:], in_=o_p[:])
            nc.scalar.dma_start(out=out_d[:], in_=o_sb[:])

    return nc


def make_in_maps(inputs):
    import ml_dtypes
    BF = ml_dtypes.bfloat16
    f32 = lambda x: np.ascontiguousarray(np.asarray(x), dtype=np.float32)

    obs = f32(inputs["obs"])
    obs_all = f32(inputs["obs_all"])
    edge = f32(inputs["edge_attrs"])
    ridxs = np.asarray(inputs["ridxs"]).astype(np.int64).reshape(-1)
    rni = np.asarray(inputs["road_neighbor_idxs"]).astype(np.int64)
    rnm = np.asarray(inputs["road_neighbor_masks"]).astype(np.int64)
    A = f32(inputs["A"])
    PA = f32(inputs["PA"])

    # parameter-only folds (second conv iteration is the only one that
    # reaches the output)
    wemb = f32(inputs["ge_wemb"])[1]
    W1, W2, W3 = wemb[:D], wemb[D:2 * D], wemb[2 * D:]
    w_obs = f32(inputs["w_obs"])
    b_obs = f32(inputs["b_obs"])
    res_w1 = f32(inputs["res_w"])[1]
    Wr = w_obs @ res_w1
    Wf1 = w_obs @ W1
    Wf2 = w_obs @ W2
    Cw = f32(inputs["ge_we"])[1] @ W3
    bias_deg = b_obs @ W1 + b_obs @ W2 + f32(inputs["ge_be"])[1] @ W3 \
        + f32(inputs["ge_bemb"])[1]
    cres = b_obs @ res_w1 + f32(inputs["res_b"])[1]
    Wst = np.concatenate([Wr, Wf1, cres[None, :], bias_deg[None, :]], axis=0)

    gw = f32(inputs["gated_w"])
    gw_a = gw[0:96]
    gw_sel = gw[96:192].reshape(M, D, 2 * D)
    W_fold = f32(inputs["w_src"]) @ gw_a
    cg_base = f32(inputs["b_src"]) @ gw_a + f32(inputs["gated_b"])
    awb = np.concatenate([f32(inputs["act_w"]),
                          f32(inputs["act_b"])[None, :]], axis=0)

    s96 = np.repeat(np.eye(M, dtype=np.float32), NQ, axis=0)
    adjfull = A + PA

    in_maps = []
    for b in range(8):
        idx = rni[ridxs[b]]
        mask = rnm[ridxs[b]].astype(np.float32)
        adjrows = adjfull[idx]                     # (6, 512)
        deg = adjrows.sum(1)                       # (6,)
        oar = obs_all[b][idx]                      # (6, 32)

        pe_h = np.zeros((P96, EKC), BF)
        ph = np.zeros((128, HKC), BF)
        pf = np.zeros((128, FKC), np.float32)

        def pute(name, arr):
            c0, r, cw = _ECOLS[name]
            pe_h[0:r, c0:c0 + cw] = np.asarray(arr).astype(BF).reshape(r, cw)

        def puth(name, arr):
            c0, r, cw = _HCOLS[name]
            ph[0:r, c0:c0 + cw] = np.asarray(arr).astype(BF).reshape(r, cw)

        def putf(name, arr):
            c0, r, cw = _FCOLS[name]
            pf[0:r, c0:c0 + cw] = np.asarray(arr, np.float32).reshape(r, cw)

        # edge rows: es96[m*NQ+q, e*JW+j2] = edge[b, idx[m], q*JW+j2, e]
        es = edge[b][idx].reshape(M, NQ, JW, ED).transpose(0, 1, 3, 2)
        pute("es96", es.reshape(P96, EQW))
        pute("adj96", adjrows.reshape(P96, JW))
        puth("oa4", obs_all[b].reshape(4, 128, OBS).transpose(1, 0, 2))
        puth("adjT", adjrows.reshape(M, 4, 128).transpose(2, 1, 0))
        gwm = (mask[:, None, None] * gw_sel).transpose(1, 0, 2)   # (D, M, 2D)
        puth("gwm1", gwm[:, :, 0:D].reshape(D, M * D))
        puth("gwm2", gwm[:, :, D:2 * D].reshape(D, M * D))
        puth("bw1", f32(inputs["base_w1"]))
        puth("bw2", f32(inputs["base_w2"]))
        puth("awb", awb)

        hp = np.concatenate([oar.T, (oar * deg[:, None]).T,
                             np.ones((1, M), np.float32), deg[None, :]], axis=0)
        putf("s96", s96)
        putf("hp66", hp)
        putf("Wst", Wst)
        putf("Wf2", Wf2)
        putf("Cw", Cw)
        cg = cg_base + ((mask - 1.0)[:, None] * gw_sel.sum(1)).sum(0)
        putf("Wfold1", np.concatenate([W_fold[:, 0:D], cg[None, 0:D]], axis=0))
        putf("Wfold2", np.concatenate([W_fold[:, D:2 * D], cg[None, D:2 * D]], axis=0))
        putf("obs1", np.concatenate([obs[b], [1.0]])[:, None])
        putf("bb1", f32(inputs["base_b1"])[:, None])
        putf("bb2", f32(inputs["base_b2"])[:, None])
        in_maps.append({"pe": pe_h, "ph": ph, "pf": pf})
    return in_maps


_CACHED = {}


def kernel(**inputs):
    from concourse.bass_utils import run_bass_kernel_spmd

    if "nc" not in _CACHED:
        _CACHED["nc"] = build_nc()
    nc = _CACHED["nc"]
    in_maps = make_in_maps(inputs)
    res = run_bass_kernel_spmd(nc, in_maps, core_ids=list(range(8)), trace=False)
    out = np.stack([np.asarray(res.results[b]["out"]).reshape(M) for b in range(8)])
    return out.astype(np.float32)


# revision 13
# speedup vs baseline: 1.0258x; 1.0258x over previous
"""Trainium2 Bass kernel for nn_ANModel (gnn_message_passing), 8-core SPMD.

kernel(**inputs) takes the FULL unsharded inputs (as produced by the
reference setup_inputs()) and returns the full (B=8, M=6) float32 output.

Strategy (one batch element per NeuronCore, no collectives):
  The reference's graph-conv loop computes res/agg from x (not x_adapt) in
  both iterations, so only the second iteration's weights reach the output;
  and the output reads x_adapt only at the M=6 rows selected by
  road_neighbor_idxs[ridxs[b]], so only those rows are computed.

  Host-side prep (sharding/gather/layout packing + parameter-only folds):
  the 6 needed edge/adjacency/obs_all rows are gathered on the host into
  two contiguous per-core tensors (bulk data + mask-folded gated/MLP
  weights in bf16, the small fp32 weight folds separately), and every
  weight-only product is precomputed in numpy.  The device runs two direct
  DMAs in (issued from the ACT engine's HW-DGE queue, which reaches its
  program earlier than the sync engine), the batch-data compute (edge
  reduce on Vector, neighborhood/fusion/MLP matmuls on PE, sigmoid/relu),
  and one DMA out — no indirect DMA, no transposes, no weight-fold matmuls
  on the critical path.
"""
import os
import sys

import numpy as np

try:
    import concourse.bass as bass
except ImportError:
    sys.path.insert(0, "/opt/trn_rl_repo")
    import concourse.bass as bass

import concourse.tile as tile
from concourse import mybir

# ---------------------------------------------------------------------------
# Workarounds for walrus builds that support only ONE sync-wait/instruction.

import concourse.tile as tile
import concourse.bass_interp as bass_interp
from concourse import mybir
from concourse.vector_clock import ScopedClock
from concourse.tile_sem_assignment import TileClockWait as _RealTCW


def _patched_drain_and_barrier(self, tick_clock, wait_clock):
    probe = self.nc.sync.drain()
    wait_clock.add_sem_waits(probe.ins, ScopedClock({None: tick_clock.global_clock}))
    si = probe.ins.sync_info
    waits = list(si.on_wait) if si and si.on_wait else []
    if len(waits) > 1:
        si.on_wait = [waits[0]]
        for w in waits[1:]:
            d = self.nc.sync.drain()
            dsi = d.ins.sync_info
            if dsi is None:
                d.ins.sync_info = mybir.SyncInfo(on_wait=[w], on_update=[])
            else:
                dsi.on_wait = [w]
    slim = os.environ.get("BASSFIX_SLIM_TAIL", "0") == "1"
    self.nc.all_engine_barrier()
    popped = self.nc._tile_sem_poison_stack.pop()
    assert popped is self._sem_poison
    self.nc.clear_and_free_semaphores(list(self.sems.allocated().values()))
    if not slim:
        self.nc.all_engine_barrier()


class _SplitWaitTCW:
    def __init__(self, tc, blocks):
        self._tc = tc
        self._blocks = blocks
        self._inner = _RealTCW(tc, blocks)

    def assign_waits(self, bb_name):
        r = self._inner.assign_waits(bb_name)
        nc = self._tc.nc
        Op = nc.isa.Opcode
        for _name, insts in self._blocks.items():
            out = []
            changed = False
            for inst in insts:
                si = getattr(inst, "sync_info", None)
                if si is not None and si.on_wait and len(si.on_wait) > 1:
                    waits = list(si.on_wait)
                    si.on_wait = [waits[-1]]
                    for w in waits[:-1]:
                        eng = nc.engines[inst.engine]
                        nop = eng._isa(Op.NEURON_ISA_TPB_OPCODE_NOP, {})
                        nop.sync_info = mybir.SyncInfo(on_wait=[w], on_update=[])
                        out.append(nop)
                    changed = True
                out.append(inst)
            if changed:
                insts[:] = out
        return r

    def add_sem_waits(self, *a, **k):
        return self._inner.add_sem_waits(*a, **k)

    def __getattr__(self, k):
        return getattr(self._inner, k)


_real_visit_isa = bass_interp._visit_InstISA


def _patched_visit_isa(isa, instruction, core_sim):
    # Treat the sequencer NOP (used by _SplitWaitTCW as a wait carrier) as a no-op.
    if instruction.isa_opcode == isa.Opcode.NEURON_ISA_TPB_OPCODE_NOP.value:
        return None
    return _real_visit_isa(isa, instruction, core_sim)


def apply():
    tile.TileContext._drain_and_barrier = _patched_drain_and_barrier
    tile.TileClockWait = _SplitWaitTCW
    bass_interp._visit_InstISA = _patched_visit_isa


apply()

# ---------------------------------------------------------------------------
# Kernel builder

F32 = mybir.dt.float32
BF16 = mybir.dt.bfloat16
AX = mybir.AxisListType
OP = mybir.AluOpType
ACT = mybir.ActivationFunctionType

N, M, ED, SRC, OBS, D, H = 512, 6, 8, 64, 32, 16, 64
NQ = 16                  # j-chunks for the edge reduce: 6*16 = 96 partitions
P96 = M * NQ             # 96
JW = N // NQ             # 32 j's per chunk
EQW = ED * JW            # 256 cols of es96 per partition row


def _mk_cols(blocks):
    cols = {}
    c = 0
    for nm, r, cw in blocks:
        cols[nm] = (c, r, cw)
        c += cw
    return cols, c


# pe (bf16, sync queue): the edge block — gates the Vector reduce
_ECOLS, EKC = _mk_cols([
    ("es96", P96, EQW),        # edge rows, (m,q) x (e,j2) layout
    ("adj96", P96, JW),        # adj rows, (m,q) x j2
])
# po (bf16, ACT queue): neighborhood-sum operands
_OCOLS, OKC = _mk_cols([
    ("oa4", 128, 4 * OBS),     # obs_all[b] as 4 chunks of (128, 32)
    ("adjT", 128, 4 * M),      # adj rows transposed, j2 x (q,m)
])
# pw (bf16, gpsimd SWDGE): weights needed late in the chain
_WCOLS, WKC = _mk_cols([
    ("gwm1", D, M * D),        # mask-scaled gated weights, "gated" half
    ("gwm2", D, M * D),        # mask-scaled gated weights, "gate" half
    ("bw1", D, H),
    ("bw2", H, H),
    ("awb", H + 1, M),         # [act_w; act_b]
])
WROWS = H + 1
# pf (f32, gpsimd SWDGE): small weight folds + batch scalars
_FCOLS, FKC = _mk_cols([
    ("s96", P96, M),           # (m,q) -> m selector
    ("hp66", 66, M),           # [oarT; oarT*deg; ones; deg]
    ("Wst", 66, D),            # [Wr; Wf1; cres; bias_deg]
    ("Wf2", OBS, D),
    ("Cw", ED, D),
    ("Wfold1", SRC + 1, D),    # [w_src @ gw_a | cg]  "gated" half
    ("Wfold2", SRC + 1, D),    # "gate" half
    ("obs1", SRC + 1, 1),      # [obs; 1]
    ("bb1", H, 1),
    ("bb2", H, 1),
])


def build_nc():
    nc = bass.Bass("TRN2", target_bir_lowering=False, debug=False)

    pe_d = nc.declare_dram_parameter("pe", [P96, EKC], BF16, isOutput=False)
    po_d = nc.declare_dram_parameter("po", [128, OKC], BF16, isOutput=False)
    pw_d = nc.declare_dram_parameter("pw", [WROWS, WKC], BF16, isOutput=False)
    pf_d = nc.declare_dram_parameter("pf", [P96, FKC], F32, isOutput=False)
    out_d = nc.declare_dram_parameter("out", [M, 1], F32, isOutput=True)

    with tile.TileContext(nc) as tc:
        with (
            tc.tile_pool(name="sb", bufs=1) as sb,
            tc.tile_pool(name="acc", bufs=1, space="PSUM") as acc,
        ):
            pe = sb.tile([P96, EKC], BF16, tag="pe")
            po = sb.tile([128, OKC], BF16, tag="po")
            pw = sb.tile([WROWS, WKC], BF16, tag="pw")
            pf = sb.tile([P96, FKC], F32, tag="pf")
            # one input DMA per engine queue so the transfers overlap; the
            # two gpsimd SWDGE transfers carry what is needed latest.
            nc.sync.dma_start(out=pe[:], in_=pe_d[:])
            nc.scalar.dma_start(out=po[:], in_=po_d[:])
            nc.gpsimd.dma_start(out=pf[:], in_=pf_d[:])
            nc.gpsimd.dma_start(out=pw[:], in_=pw_d[:])

            def E_(name):
                c0, r, cw = _ECOLS[name]
                return pe[0:r, c0:c0 + cw]

            def O_(name):
                c0, r, cw = _OCOLS[name]
                return po[0:r, c0:c0 + cw]

            def W_(name):
                c0, r, cw = _WCOLS[name]
                return pw[0:r, c0:c0 + cw]

            def F_(name):
                c0, r, cw = _FCOLS[name]
                return pf[0:r, c0:c0 + cw]

            # h3 gets a constant-1 row appended so the action head is a
            # single fused [act_w; act_b] matmul.
            h3 = sb.tile([H + 1, 1], BF16, tag="h3")
            nc.gpsimd.memset(h3[H:H + 1, 0:1], 1.0)

            es96 = E_("es96")
            adj96 = E_("adj96")

            # ---- edge reduce: er[(m,q), e] = sum_j2 es*adj ----
            prod = sb.tile([P96, EQW], F32, tag="prod")
            in0 = bass.AP(tensor=es96.tensor, offset=es96.offset,
                          ap=[es96.ap[0], [JW, ED], [1, JW]])
            in1 = bass.AP(tensor=adj96.tensor, offset=adj96.offset,
                          ap=[adj96.ap[0], [0, ED], [1, JW]])
            out0 = bass.AP(tensor=prod[:].tensor, offset=prod[:].offset,
                           ap=[prod[:].ap[0], [JW, ED], [1, JW]])
            erq = sb.tile([P96, ED], F32, tag="erq")
            pr = prod[:]
            red_in = bass.AP(tensor=pr.tensor, offset=pr.offset,
                             ap=[pr.ap[0], [JW, ED], [1, JW]])
            with tc.high_priority():
                nc.vector.tensor_tensor(out=out0, in0=in0, in1=in1, op=OP.mult)
                nc.vector.tensor_reduce(out=erq[:], in_=red_in, axis=AX.X, op=OP.add)

            # ---- PE: gated-fusion head start, S^T, er^T, zf^T, MLP ----
            g1_p = acc.tile([D, 1], F32, tag="g1_p")
            g2_p = acc.tile([D, 1], F32, tag="g2_p")
            nc.tensor.matmul(out=g1_p[:], lhsT=F_("Wfold1"), rhs=F_("obs1"),
                             start=True, stop=False)
            nc.tensor.matmul(out=g2_p[:], lhsT=F_("Wfold2"), rhs=F_("obs1"),
                             start=True, stop=False)
            zfT_p = acc.tile([D, M], F32, tag="zfT_p")
            nc.tensor.matmul(out=zfT_p[:], lhsT=F_("Wst"), rhs=F_("hp66"),
                             start=True, stop=False)
            sT_p = acc.tile([OBS, M], F32, tag="sT_p")
            oa4 = O_("oa4")
            adjT = O_("adjT")
            for c in range(4):
                nc.tensor.matmul(out=sT_p[:], lhsT=oa4[:, c * OBS:(c + 1) * OBS],
                                 rhs=adjT[:, c * M:(c + 1) * M],
                                 start=(c == 0), stop=(c == 3))
            erT_p = acc.tile([ED, M], F32, tag="erT_p")
            nc.tensor.matmul(out=erT_p[:], lhsT=erq[:], rhs=F_("s96"),
                             start=True, stop=True)
            # sT copy on ACT (first ACT compute op -> also absorbs the
            # activation-table load into its operand-wait window),
            # erT copy on the idle Vector engine: the two run in parallel.
            sT = sb.tile([OBS, M], F32, tag="sT")
            nc.scalar.copy(out=sT[:], in_=sT_p[:])
            erT = sb.tile([ED, M], F32, tag="erT")
            nc.vector.tensor_copy(out=erT[:], in_=erT_p[:])
            nc.tensor.matmul(out=zfT_p[:], lhsT=F_("Wf2"), rhs=sT[:],
                             start=False, stop=False)
            nc.tensor.matmul(out=zfT_p[:], lhsT=F_("Cw"), rhs=erT[:],
                             start=False, stop=True)
            rT = sb.tile([D, M], BF16, tag="rT")
            nc.vector.tensor_scalar(out=rT[:], in0=zfT_p[:], scalar1=0.0,
                                    scalar2=None, op0=OP.max)
            gwm1 = W_("gwm1")
            gwm2 = W_("gwm2")
            for m in range(M):
                nc.tensor.matmul(out=g2_p[:], lhsT=gwm2[:, m * D:(m + 1) * D],
                                 rhs=rT[:, m:m + 1],
                                 start=False, stop=(m == M - 1))
            for m in range(M):
                nc.tensor.matmul(out=g1_p[:], lhsT=gwm1[:, m * D:(m + 1) * D],
                                 rhs=rT[:, m:m + 1],
                                 start=False, stop=(m == M - 1))
            sig = sb.tile([D, 1], F32, tag="sig")
            nc.scalar.activation(out=sig[:], in_=g2_p[:], func=ACT.Sigmoid)
            h = sb.tile([D, 1], BF16, tag="h")
            nc.vector.tensor_mul(out=h[:], in0=g1_p[:], in1=sig[:])

            h2_p = acc.tile([H, 1], F32, tag="h2_p")
            nc.tensor.matmul(out=h2_p[:], lhsT=W_("bw1"), rhs=h[:], start=True, stop=True)
            h2 = sb.tile([H, 1], BF16, tag="h2")
            nc.vector.tensor_scalar(out=h2[:], in0=h2_p[:], scalar1=F_("bb1")[:, 0:1],
                                    scalar2=0.0, op0=OP.add, op1=OP.max)
            h3_p = acc.tile([H, 1], F32, tag="h3_p")
            nc.tensor.matmul(out=h3_p[:], lhsT=W_("bw2"), rhs=h2[:], start=True, stop=True)
            nc.vector.tensor_scalar(out=h3[0:H, 0:1], in0=h3_p[:],
                                    scalar1=F_("bb2")[:, 0:1],
                                    scalar2=0.0, op0=OP.add, op1=OP.max)
            o_p = acc.tile([M, 1], F32, tag="o_p")
            nc.tensor.matmul(out=o_p[:], lhsT=W_("awb"), rhs=h3[:], start=True, stop=True)
            o_sb = sb.tile([M, 1], F32, tag="o_sb")
            nc.vector.tensor_copy(out=o_sb[:], in_=o_p[:])
            nc.gpsimd.dma_start(out=out_d[:], in_=o_sb[:])

    return nc


def make_in_maps(inputs):
    import ml_dtypes
    BF = ml_dtypes.bfloat16
    f32 = lambda x: np.ascontiguousarray(np.asarray(x), dtype=np.float32)

    obs = f32(inputs["obs"])
    obs_all = f32(inputs["obs_all"])
    edge = f32(inputs["edge_attrs"])
    ridxs = np.asarray(inputs["ridxs"]).astype(np.int64).reshape(-1)
    rni = np.asarray(inputs["road_neighbor_idxs"]).astype(np.int64)
    rnm = np.asarray(inputs["road_neighbor_masks"]).astype(np.int64)
    A = f32(inputs["A"])
    PA = f32(inputs["PA"])

    # parameter-only folds (second conv iteration is the only one that
    # reaches the output)
    wemb = f32(inputs["ge_wemb"])[1]
    W1, W2, W3 = wemb[:D], wemb[D:2 * D], wemb[2 * D:]
    w_obs = f32(inputs["w_obs"])
    b_obs = f32(inputs["b_obs"])
    res_w1 = f32(inputs["res_w"])[1]
    Wr = w_obs @ res_w1
    Wf1 = w_obs @ W1
    Wf2 = w_obs @ W2
    Cw = f32(inputs["ge_we"])[1] @ W3
    bias_deg = b_obs @ W1 + b_obs @ W2 + f32(inputs["ge_be"])[1] @ W3 \
        + f32(inputs["ge_bemb"])[1]
    cres = b_obs @ res_w1 + f32(inputs["res_b"])[1]
    Wst = np.concatenate([Wr, Wf1, cres[None, :], bias_deg[None, :]], axis=0)

    gw = f32(inputs["gated_w"])
    gw_a = gw[0:96]
    gw_sel = gw[96:192].reshape(M, D, 2 * D)
    W_fold = f32(inputs["w_src"]) @ gw_a
    cg_base = f32(inputs["b_src"]) @ gw_a + f32(inputs["gated_b"])
    awb = np.concatenate([f32(inputs["act_w"]),
                          f32(inputs["act_b"])[None, :]], axis=0)

    s96 = np.repeat(np.eye(M, dtype=np.float32), NQ, axis=0)
    adjfull = A + PA

    in_maps = []
    for b in range(8):
        idx = rni[ridxs[b]]
        mask = rnm[ridxs[b]].astype(np.float32)
        adjrows = adjfull[idx]                     # (6, 512)
        deg = adjrows.sum(1)                       # (6,)
        oar = obs_all[b][idx]                      # (6, 32)

        pe_h = np.zeros((P96, EKC), BF)
        po_h = np.zeros((128, OKC), BF)
        pw_h = np.zeros((WROWS, WKC), BF)
        pf_h = np.zeros((P96, FKC), np.float32)

        def put(buf, cols, name, arr, dt):
            c0, r, cw = cols[name]
            buf[0:r, c0:c0 + cw] = np.asarray(arr).astype(dt).reshape(r, cw)

        # edge rows: es96[m*NQ+q, e*JW+j2] = edge[b, idx[m], q*JW+j2, e]
        es = edge[b][idx].reshape(M, NQ, JW, ED).transpose(0, 1, 3, 2)
        put(pe_h, _ECOLS, "es96", es.reshape(P96, EQW), BF)
        put(pe_h, _ECOLS, "adj96", adjrows.reshape(P96, JW), BF)
        put(po_h, _OCOLS, "oa4", obs_all[b].reshape(4, 128, OBS).transpose(1, 0, 2), BF)
        put(po_h, _OCOLS, "adjT", adjrows.reshape(M, 4, 128).transpose(2, 1, 0), BF)
        gwm = (mask[:, None, None] * gw_sel).transpose(1, 0, 2)   # (D, M, 2D)
        put(pw_h, _WCOLS, "gwm1", gwm[:, :, 0:D].reshape(D, M * D), BF)
        put(pw_h, _WCOLS, "gwm2", gwm[:, :, D:2 * D].reshape(D, M * D), BF)
        put(pw_h, _WCOLS, "bw1", f32(inputs["base_w1"]), BF)
        put(pw_h, _WCOLS, "bw2", f32(inputs["base_w2"]), BF)
        put(pw_h, _WCOLS, "awb", awb, BF)

        put(pf_h, _FCOLS, "s96", s96, np.float32)
        hp = np.concatenate([oar.T, (oar * deg[:, None]).T,
                             np.ones((1, M), np.float32), deg[None, :]], axis=0)
        put(pf_h, _FCOLS, "hp66", hp, np.float32)
        put(pf_h, _FCOLS, "Wst", Wst, np.float32)
        put(pf_h, _FCOLS, "Wf2", Wf2, np.float32)
        put(pf_h, _FCOLS, "Cw", Cw, np.float32)
        cg = cg_base + ((mask - 1.0)[:, None] * gw_sel.sum(1)).sum(0)
        put(pf_h, _FCOLS, "Wfold1",
            np.concatenate([W_fold[:, 0:D], cg[None, 0:D]], axis=0), np.float32)
        put(pf_h, _FCOLS, "Wfold2",
            np.concatenate([W_fold[:, D:2 * D], cg[None, D:2 * D]], axis=0), np.float32)
        put(pf_h, _FCOLS, "obs1",
            np.concatenate([obs[b], [1.0]])[:, None], np.float32)
        put(pf_h, _FCOLS, "bb1", f32(inputs["base_b1"])[:, None], np.float32)
        put(pf_h, _FCOLS, "bb2", f32(inputs["base_b2"])[:, None], np.float32)
        in_maps.append({"pe": pe_h, "po": po_h, "pw": pw_h, "pf": pf_h})
    return in_maps


_CACHED = {}


def kernel(**inputs):
    from concourse.bass_utils import run_bass_kernel_spmd

    if "nc" not in _CACHED:
        _CACHED["nc"] = build_nc()
    nc = _CACHED["nc"]
    in_maps = make_in_maps(inputs)
    res = run_bass_kernel_spmd(nc, in_maps, core_ids=list(range(8)), trace=False)
    out = np.stack([np.asarray(res.results[b]["out"]).reshape(M) for b in range(8)])
    return out.astype(np.float32)
